# revision 32
# baseline (speedup 1.0000x reference)
"""AnchorDML Trainium2 kernel: 8-core SPMD, data-parallel over x rows with
sharded anchor encoding + AllGather of encoded anchors.

Problem (hardcoded):
    N, M, D, C = 8192, 4096, 512, 100
    xe = mish(mish(x @ W1 + b1) @ W2 + b2)          [N, D]
    se = mish(mish(samples @ W1 + b1) @ W2 + b2)    [M, D]
    dist = sqrt(max(|xe|^2 + |se|^2 - 2 xe@se.T, 0))  [N, M]
    out = log_softmax(tanh(dist @ Wp + bp), axis=1)   [N, C]

Sharding: core g handles x rows [1024g, 1024(g+1)) and encodes anchors
[512g, 512(g+1)); encoded (scaled) anchors + |se|^2 are AllGathered.

Design (trace-driven, vs the 213us baseline):
 - fp8(e4m3) distance GEMM in DoubleRow perf mode: seA and xe are stored
   as fp8 [128, 4k, cols]; each d2 tile is 2 matmuls contracting 256.
   |xe|^2 / |se|^2 are computed FROM the fp8-rounded values so d2 is
   exactly the squared distance of the fp8-perturbed points.
 - one AllGather moves [512, 258]-bf16 per rank: cols 0:256 hold the fp8
   seA pairs, cols 256:258 hold anchor r's fp32 |se|^2/... packed per
   row, so a single collective carries both and every gathered-side
   access pattern stays <= 3 dims.
 - a tiny warmup AllGather triggers ~10us in: it resolves the CC entry
   barrier and wakes the ncfw firmware, which measured 2.6x faster
   transfer + less start delay on the real collective.
 - the anchor->trigger chain is kept minimal: eTs/W1 loads head two DMA
   queues, later loads (W2, eTx, Wp) are semaphore-gated so they don't
   steal HBM bandwidth, sqse runs on ACT's Square (co-resident with
   Tanh: no table load), and dep helpers keep all anchor work ahead of
   x-side work on every engine.
 - x-side layer 2 is split into 512-row halves so the first half's xe
   and |xe|^2 are ready when the collective lands.
 - the main loop is rc-major (each half sweeps all 32 anchor tiles,
   2 fp8 DoubleRow matmuls each); the bias add drains PSUM into SBUF on
   DVE so the ACT sqrt pass is decoupled from the PE stream; each
   half's perceptron PSUM + transposes overlap the other half's sweep.
"""
import numpy as np
import ml_dtypes
from concourse import bass, bacc, tile, mybir, bass_utils, masks

N, M, D, C = 8192, 4096, 512, 100
NCORES = 8
RPC = N // NCORES      # 1024 x-rows per core
MPC = M // NCORES      # 512 anchors encoded per core
KD = D // 128          # 4 contraction chunks of 128
NMT = M // 128         # 32 anchor tiles in the distance matmul
NRC = RPC // 512       # 2 row-chunks of 512
TPG = MPC // 128       # 4 anchor tiles per gathered rank
AGW = MPC // 2         # ag row width in bf16 units (fp8 pairs)
AGR = D + 4            # 512 fp8 anchor rows + 4 rows of packed fp32 s2
LAG = 3                # zT matmul trails the d2 tiles by LAG tiles

F32 = mybir.dt.float32
F32R = mybir.dt.float32r
BF16 = mybir.dt.bfloat16
FP8 = mybir.dt.float8e4
AF = mybir.ActivationFunctionType
ALU = mybir.AluOpType
DR = mybir.MatmulPerfMode.DoubleRow


def _patched_tables(arch):
    """Subset the ACT table sets (keeping dict order — act_func_set_id is
    positional) so Exp/Ln resolve only to natural_log_exp_and_others and
    Tanh only to exp_and_others. The default first-match choice alternates
    exp_and_others <-> natural_log on every exp/ln pair, paying a 1.3us
    table load each time."""
    from concourse.hw_specs import get_activation_tables as orig
    out = {}
    for name, s in orig(arch).items():
        s = set(s)
        if name != "natural_log_exp_and_others":
            s.discard(AF.Exp)
            s.discard(AF.Ln)
            # keep Copy co-resident with Exp/Ln so the startup const
            # copies don't force an extra table load before the encoder
            if name != "sqrt_and_others":
                s.discard(AF.Copy)
        if name != "exp_and_others":
            s.discard(AF.Tanh)
            s.discard(AF.Square)
        out[name] = s
    return out


def build_kernel():
    bacc.get_activation_tables = _patched_tables
    nc = bacc.Bacc("TRN2", target_bir_lowering=False, debug=False,
                   num_devices=NCORES)

    eT = nc.dram_tensor("eT", [D, MPC + RPC], BF16, kind="ExternalInput")
    W1 = nc.dram_tensor("W1", [D, D], BF16, kind="ExternalInput")
    W2 = nc.dram_tensor("W2", [D, D], BF16, kind="ExternalInput")
    b1 = nc.dram_tensor("b1", [D, 1], F32, kind="ExternalInput")
    b2 = nc.dram_tensor("b2", [D, 1], F32, kind="ExternalInput")
    Wp = nc.dram_tensor("Wp", [M, C], F32, kind="ExternalInput")
    bp = nc.dram_tensor("bp", [1, C], F32, kind="ExternalInput")
    out = nc.dram_tensor("out", [RPC, C], F32, kind="ExternalOutput")

    with tile.TileContext(nc) as tc:
        _body(tc, eT, W1, W2, b1, b2, Wp, bp, out)

    nc.compile()
    return nc


def _body(tc, eT, W1, W2, b1, b2, Wp, bp, out):
    nc = tc.nc
    with (
        tc.tile_pool(name="const", bufs=1) as const,
        tc.tile_pool(name="wpool", bufs=1) as wpool,
        tc.tile_pool(name="spool", bufs=1) as spool,
        tc.tile_pool(name="xpool", bufs=1) as xpool,
        tc.tile_pool(name="gpool", bufs=1) as gpool,
        tc.tile_pool(name="mpool", bufs=2) as mpool,
        tc.tile_pool(name="dpool", bufs=1) as dpool,
        tc.tile_pool(name="zpool", bufs=2) as zpool,
        tc.tile_pool(name="ps", bufs=1, space="PSUM") as ps,
        tc.tile_pool(name="psz", bufs=1, space="PSUM") as psz,
        tc.tile_pool(name="dram", bufs=1, space="DRAM") as dram,
    ):
        # ---- warmup collective staging heads the pool queue (128B, in
        # flight by ~10us even under input-load traffic) ----
        warm_sb = const.tile([1, 64], BF16)
        nc.gpsimd.memset(warm_sb[:], 1.0)
        warm_in = dram.tile([1, 64], BF16)
        warm_out = dram.tile([NCORES, 64], BF16, addr_space="Shared")
        nc.gpsimd.dma_start(warm_in[:], warm_sb[:])
        nc.gpsimd.collective_compute(
            "AllGather", ALU.bypass,
            replica_groups=[list(range(NCORES))],
            ins=[warm_in.opt()], outs=[warm_out.opt()])

        # ---- input loads: the two tensors gating the first anchor
        # matmul (eTs, W1) head the sync and pool queues so their ~1MB
        # gets the full HBM bandwidth; W2/eTx/Wp issue is gated (helpers
        # added below) until the anchor encode is underway. ----
        eTs_sb = spool.tile([128, KD, MPC], BF16)
        for h in range(2):
            nc.sync.dma_start(
                eTs_sb[:, 2 * h:2 * h + 2, :],
                eT[256 * h:256 * (h + 1), :MPC].rearrange(
                    "(k p) m -> p k m", p=128))
        W1_sb = wpool.tile([128, KD, D], BF16)
        for h in range(2):
            nc.gpsimd.dma_start(
                W1_sb[:, 2 * h:2 * h + 2, :],
                W1[256 * h:256 * (h + 1), :].rearrange(
                    "(k p) d -> p k d", p=128))
        b1c_sb = wpool.tile([128, KD], F32)
        nc.gpsimd.dma_start(b1c_sb[:],
                            b1[:].rearrange("(k p) o -> p (k o)", p=128))
        b2c_sb = wpool.tile([128, KD], F32)
        nc.gpsimd.dma_start(b2c_sb[:],
                            b2[:].rearrange("(k p) o -> p (k o)", p=128))

        W2_sb = wpool.tile([128, KD, D], BF16)
        w2_dma = nc.sync.dma_start(W2_sb[:, :, :],
                                   W2[:].rearrange("(k p) d -> p k d", p=128))
        eTx_sb = xpool.tile([128, KD, RPC], BF16)
        etx_dmas = []
        for rc in range(NRC):
            etx_dmas.append(nc.sync.dma_start(
                eTx_sb[:, :, 512 * rc:512 * (rc + 1)],
                eT[:, MPC + 512 * rc:MPC + 512 * (rc + 1)].rearrange(
                    "(k p) m -> p k m", p=128)))

        # ---- constants (pool queue, before the gated Wp load) ----
        ident = const.tile([C, C], F32)
        masks.make_identity(nc, ident[:])
        ones_col = const.tile([128, 1], BF16)    # lhsT for row-sum matmuls
        nc.gpsimd.memset(ones_col[:], 1.0)
        onesr_f32 = const.tile([1, 512], F32)
        nc.gpsimd.memset(onesr_f32[:], 1.0)
        ones512 = const.tile([1, 512], F32R)     # rhs/lhsT for rank-1 terms
        nc.scalar.activation(ones512[:], onesr_f32[:], AF.Copy)
        bp_sb = wpool.tile([1, C], F32R)
        nc.gpsimd.dma_start(bp_sb[:], bp[:].bitcast(F32R))

        Wp_sb = wpool.tile([128, NMT, C], F32R)
        wp_dma = nc.gpsimd.dma_start(
            Wp_sb[:, :, :],
            Wp[:].bitcast(F32R).rearrange("(t p) c -> p t c", p=128))

        W1_ks = [W1_sb[:, k, :] for k in range(KD)]
        W2_ks = [W2_sb[:, k, :] for k in range(KD)]
        eTs_ks = [eTs_sb[:, k, :] for k in range(KD)]
        eTx_ks = [eTx_sb[:, k, :] for k in range(KD)]

        def enc_phase(dst, dst_off, Wks, bcol, src_ks, src_off, width,
                      vscale=None):
            """dst[:, :, dst_off:dst_off+width] = mish(src.T @ W + b) in
            dst's dtype (bf16 hidden / fp8 final). v is staged (with bias)
            so psum recycles fast; sp=ln(1+e^v) lands in a bf16 temp; tanh
            + the v*t multiply are batched over the whole phase."""
            nw = width // 512
            first_exp, first_vadd, first_mm = [], [], []
            vstage = mpool.tile([128, KD, width], BF16, tag="vstage")
            tstage = mpool.tile([128, KD, width], BF16, tag="tstage")
            for w in range(nw):
                ssl = slice(src_off + 512 * w, src_off + 512 * (w + 1))
                for f in range(KD):
                    vps = ps.tile([128, 512], F32, tag="mm", bufs=4)
                    for k in range(KD):
                        mm_i = nc.tensor.matmul(vps[:],
                                                Wks[k][:, 128 * f:128 * (f + 1)],
                                                src_ks[k][:, ssl],
                                                start=(k == 0),
                                                stop=(k == KD - 1))
                        if not first_mm:
                            first_mm.append(mm_i)
                    u = mpool.tile([128, 512], BF16, tag="mu", bufs=4)
                    e_i = nc.scalar.activation(u[:], vps[:], AF.Exp,
                                               bias=bcol[:, f:f + 1])
                    if not first_exp:
                        first_exp.append(e_i)
                    if vscale is None:
                        v_i = nc.vector.tensor_scalar_add(
                            vstage[:, f, 512 * w:512 * (w + 1)], vps[:],
                            bcol[:, f:f + 1])
                    else:
                        v_i = nc.vector.tensor_scalar(
                            vstage[:, f, 512 * w:512 * (w + 1)], vps[:],
                            bcol[:, f:f + 1], vscale,
                            op0=ALU.add, op1=ALU.mult)
                    if not first_vadd:
                        first_vadd.append(v_i)
                    nc.scalar.activation(tstage[:, f, 512 * w:512 * (w + 1)],
                                         u[:], AF.Ln, bias=1.0)
            dsl = slice(dst_off, dst_off + width)
            tanh_i = nc.scalar.activation(tstage[:, :, :width],
                                          tstage[:, :, :width], AF.Tanh)
            mult_i = nc.vector.tensor_tensor(dst[:, :, dsl],
                                             vstage[:, :, :width],
                                             tstage[:, :, :width], op=ALU.mult)
            return {"first_exp": first_exp[0], "first_vadd": first_vadd[0],
                    "first_mm": first_mm[0], "tanh": tanh_i, "mult": mult_i}

        # ---- anchor columns first, both layers, so the AllGather can be
        # issued as early as possible; the -2 distance scaling is folded
        # into the layer-2 mish multiply so it emits seA = -2*se (exact
        # in fp8: power-of-2 scale) ----
        h_se = spool.tile([128, KD, MPC], BF16)
        hse_ks = [h_se[:, k, :] for k in range(KD)]
        se1_h = enc_phase(h_se, 0, W1_ks, b1c_sb, eTs_ks, 0, MPC)
        seA_sb = spool.tile([128, KD, MPC], FP8)
        se2_h = enc_phase(seA_sb, 0, W2_ks, b2c_sb, hse_ks, 0, MPC,
                          vscale=-2.0)
        # gate the bandwidth-hungry loads behind the first anchor matmul
        tile.add_dep_helper(w2_dma.ins, se1_h["first_mm"].ins, sync=True,
                            reason="W2 issue after anchor L1 starts")
        for e in etx_dmas:
            tile.add_dep_helper(e.ins, se1_h["first_mm"].ins, sync=True,
                                reason="eTx issue after anchor L1 starts")
        tile.add_dep_helper(wp_dma.ins, se2_h["first_mm"].ins, sync=True,
                            reason="Wp issue after anchor L2 starts")

        # s2 row = sum_d seA^2 / 4 (from the fp8-rounded values, so d2 is
        # exactly the squared distance of the perturbed points). Square
        # runs on ACT (co-resident with Tanh: no table load).
        sqse_sb = spool.tile([128, KD, MPC], BF16, tag="h_se")
        sq_se_i = nc.scalar.activation(sqse_sb[:, :, :], seA_sb[:, :, :],
                                       AF.Square)
        s2ps = ps.tile([1, 512], F32, tag="tr", bufs=2)
        for k in range(KD):
            s2_mm_i = nc.tensor.matmul(s2ps[:], ones_col[:], sqse_sb[:, k, :],
                                       start=(k == 0), stop=(k == KD - 1))
        s2row_sb = spool.tile([1, MPC], F32)
        s2row_i = nc.vector.tensor_scalar_mul(s2row_sb[:], s2ps[:], 0.25)

        # one collective: [seA (fp8 as bf16 pairs, 512 rows) ; s2 (fp32
        # packed as 4 rows)] — row-appended so the gathered per-rank seA
        # reads stay fully contiguous (a strided row read costs ~6us of
        # descriptor-issue time)
        ag_in = dram.tile([AGR, AGW], BF16)
        ag_out = dram.tile([NCORES * AGR, AGW], BF16, addr_space="Shared")
        nc.sync.dma_start(
            ag_in[:D, :].rearrange("(k p) m -> p k m", p=128),
            seA_sb[:, :, :].bitcast(BF16))
        nc.sync.dma_start(
            ag_in[D:AGR, :].rearrange("(o a) m -> o (a m)", o=1),
            s2row_sb[:].bitcast(BF16))
        nc.gpsimd.collective_compute(
            "AllGather", ALU.bypass,
            replica_groups=[list(range(NCORES))],
            ins=[ag_in.opt()], outs=[ag_out.opt()])

        # ---- x columns overlap the AllGather, both layers split into
        # 512-row halves so the first half's xe and |xe|^2 are ready the
        # moment the collective lands. Ordering hints keep the
        # anchor->AllGather chain ahead of x-side work on every engine. ----
        h_xe = xpool.tile([128, KD, RPC], BF16)
        hxe_ks = [h_xe[:, k, :] for k in range(KD)]
        xe_sb = xpool.tile([128, KD, RPC], FP8)
        x2row_sb = xpool.tile([1, RPC], F32R)
        x2b_sb = xpool.tile([128, RPC], F32)
        def x2_phase(rc):
            """|xe|^2 row + broadcast tile for one 512-row half."""
            rsl = slice(512 * rc, 512 * (rc + 1))
            sqxe = xpool.tile([128, KD, 512], BF16, tag=f"sqxe{rc}")
            nc.vector.tensor_tensor(sqxe[:, :, :], xe_sb[:, :, rsl],
                                    xe_sb[:, :, rsl], op=ALU.mult)
            xps = ps.tile([1, 512], F32, tag="tr", bufs=2)
            for k in range(KD):
                nc.tensor.matmul(xps[:], ones_col[:], sqxe[:, k, :],
                                 start=(k == 0), stop=(k == KD - 1))
            nc.vector.tensor_copy(x2row_sb[:, rsl], xps[:])
            xbs = ps.tile([128, 512], F32, tag="mm", bufs=4)
            nc.tensor.matmul(xbs[:], ones512[:, :128], x2row_sb[:, rsl],
                             start=True, stop=True)
            nc.vector.tensor_copy(x2b_sb[:, rsl], xbs[:])

        xe1_hs, xe2_hs = [], []
        xe1_hs.append(enc_phase(h_xe, 0, W1_ks, b1c_sb, eTx_ks, 0, 512))
        xe2_hs.append(enc_phase(xe_sb, 0, W2_ks, b2c_sb, hxe_ks, 0, 512))
        x2_phase(0)
        tile.add_dep_helper(xe1_hs[0]["first_exp"].ins, sq_se_i.ins, sync=False,
                            reason="anchor ACT chain before x-side ACT")
        tile.add_dep_helper(xe1_hs[0]["first_vadd"].ins, s2row_i.ins, sync=False,
                            reason="anchor DVE chain before x-side DVE")
        tile.add_dep_helper(xe1_hs[0]["first_mm"].ins, s2_mm_i.ins, sync=False,
                            reason="anchor PE chain + s2 sums before x-side PE")

        # ---- load gathered anchors: per-rank seA descriptors alternate
        # between the sync and pool queues (multiple descriptors spread
        # across DMA engines; a single big one serializes on one engine).
        # The tiny s2 descriptor and rank 0 head the queues: they gate
        # the first distance tile. ----
        s2c_sb = gpool.tile([128, NCORES, TPG], F32)
        seAg_sb = gpool.tile([128, NCORES, KD, MPC], FP8)
        for g in range(NCORES):
            q = nc.sync if g % 2 == 0 else nc.gpsimd
            descs = [
                (s2c_sb[:, g, :],
                 ag_out[AGR * g + D:AGR * (g + 1), :].bitcast(F32)
                 .rearrange("a p -> p a")),
                (seAg_sb[:, g, :, :].bitcast(BF16),
                 ag_out[AGR * g:AGR * g + D, :].rearrange(
                     "(k p) m -> p k m", p=128)),
            ]
            # rank 0 gates the first tile: its s2 first; later ranks'
            # (bigger) seA payloads head their queue slots instead
            for dst, src in (descs if g < 2 else descs[::-1]):
                q.dma_start(dst, src)

        # ---- main fused loop, rc-major: each 512-row half sweeps all 32
        # anchor tiles (2 fp8 DoubleRow matmuls each, contraction 256),
        # drains its perceptron PSUM, and runs its own log-softmax
        # epilogue + output DMA while the other half still sweeps. The
        # zT matmul for tile t is emitted after the d2 group of tile
        # t+LAG so the in-order PE stream never waits on the sqrt pass. ----
        def sweep(rc):
            rsl = slice(512 * rc, 512 * (rc + 1))
            ztp = psz.tile([C, 512], F32, tag="zt", bufs=2)
            nc.tensor.matmul(ztp[:], bp_sb[:], ones512[:],
                             start=True, stop=False, skip_group_check=True)
            dist_tiles = {}
            for t in range(NMT):
                g, tl = divmod(t, TPG)
                d2ps = ps.tile([128, 512], F32, tag="mm", bufs=4)
                for q in range(2):
                    nc.tensor.matmul(
                        d2ps[:],
                        seAg_sb[:, g, 2 * q:2 * q + 2, 128 * tl:128 * (tl + 1)],
                        xe_sb[:, 2 * q:2 * q + 2, rsl],
                        start=(q == 0), stop=(q == 1), perf_mode=DR)
                # d2 += s2[m] (per-partition) + x2[r] (broadcast row), fp32
                nc.vector.scalar_tensor_tensor(
                    d2ps[:], d2ps[:], s2c_sb[:, g, tl:tl + 1],
                    x2b_sb[:, rsl], op0=ALU.add, op1=ALU.add)
                distT = dpool.tile([128, 512], F32R, tag="dist", bufs=5)
                sq_i = nc.scalar.activation(distT[:], d2ps[:], AF.Sqrt)
                if rc == 0 and t == 0:
                    tile.add_dep_helper(
                        sq_i.ins, xe2_hs[0]["tanh"].ins, sync=False,
                        reason="sqrt table load after this half's encoder ACT")
                dist_tiles[t] = distT
                if t >= LAG:
                    nc.tensor.matmul(ztp[:], Wp_sb[:, t - LAG, :],
                                     dist_tiles.pop(t - LAG)[:],
                                     start=False, stop=False,
                                     skip_group_check=True)
            for t in range(NMT - LAG, NMT):
                nc.tensor.matmul(ztp[:], Wp_sb[:, t, :],
                                 dist_tiles.pop(t)[:],
                                 start=False, stop=(t == NMT - 1),
                                 skip_group_check=True)
            # drain this half: bias'd zT -> sbuf, transpose to row-major,
            # then tanh + log-softmax (tanh output is in [-1,1] so no
            # max-subtraction is needed) and the output DMA
            zt_sb = zpool.tile([C, 512], F32, bufs=2, tag="ztsb")
            nc.vector.tensor_copy(zt_sb[:], ztp[:])
            zpre_sb = zpool.tile([128, 4, C], BF16, bufs=2, tag="zpre")
            for j in range(4):
                ztr = ps.tile([128, C], F32, tag="tr", bufs=2)
                nc.tensor.matmul(ztr[:], zt_sb[:, 128 * j:128 * (j + 1)],
                                 ident[:], is_transpose=True)
                nc.vector.tensor_copy(zpre_sb[:, j, :], ztr[:])
            zth_sb = zpool.tile([128, 4, C], BF16, bufs=2, tag="zth")
            nc.scalar.activation(zth_sb[:, :, :], zpre_sb[:, :, :], AF.Tanh)
            e_sb = zpool.tile([128, 4, C], BF16, bufs=2, tag="esb")
            nc.scalar.activation(e_sb[:, :, :], zth_sb[:, :, :], AF.Exp)
            ssum = zpool.tile([128, 4], F32, bufs=2, tag="ssum")
            nc.vector.tensor_reduce(ssum[:], e_sb[:, :, :],
                                    axis=mybir.AxisListType.X, op=ALU.add)
            lns = zpool.tile([128, 4], F32, bufs=2, tag="lns")
            nc.scalar.activation(lns[:], ssum[:], AF.Ln)
            o_sb = zpool.tile([128, 4, C], F32, bufs=2, tag="osb")
            for j in range(4):
                nc.vector.tensor_scalar(o_sb[:, j, :], zth_sb[:, j, :],
                                        lns[:, j:j + 1], None,
                                        op0=ALU.subtract)
            nc.sync.dma_start(
                out[512 * rc:512 * (rc + 1), :].rearrange(
                    "(j p) c -> p j c", p=128),
                o_sb[:, :, :])

        sweep(0)
        # the rc1 encode is emitted after the rc0 sweep: its matmuls/ACT
        # then overlap the sweep instead of blocking the in-order PE
        # queue ahead of it
        xe1_hs.append(enc_phase(h_xe, 512, W1_ks, b1c_sb, eTx_ks, 512, 512))
        xe2_hs.append(enc_phase(xe_sb, 512, W2_ks, b2c_sb, hxe_ks, 512, 512))
        x2_phase(1)
        sweep(1)


_NC_CACHE = None


def _get_nc():
    global _NC_CACHE
    if _NC_CACHE is None:
        _NC_CACHE = build_kernel()
    return _NC_CACHE


def make_in_maps(x, samples, W1, b1, W2, b2, Wp, bp):
    bf = ml_dtypes.bfloat16
    x = np.asarray(x, dtype=np.float32)
    samples = np.asarray(samples, dtype=np.float32)
    W1b = np.ascontiguousarray(np.asarray(W1, dtype=np.float32).astype(bf))
    W2b = np.ascontiguousarray(np.asarray(W2, dtype=np.float32).astype(bf))
    Wpc = np.ascontiguousarray(np.asarray(Wp, dtype=np.float32))
    b1c = np.ascontiguousarray(np.asarray(b1, dtype=np.float32).reshape(D, 1))
    b2c = np.ascontiguousarray(np.asarray(b2, dtype=np.float32).reshape(D, 1))
    bpc = np.ascontiguousarray(np.asarray(bp, dtype=np.float32).reshape(1, C))
    in_maps = []
    for g in range(NCORES):
        sT_g = samples[MPC * g:MPC * (g + 1), :].T
        xT_g = x[RPC * g:RPC * (g + 1), :].T
        eT_g = np.concatenate([sT_g, xT_g], axis=1).astype(bf)
        in_maps.append({
            "eT": np.ascontiguousarray(eT_g),
            "W1": W1b, "W2": W2b, "b1": b1c, "b2": b2c,
            "Wp": Wpc, "bp": bpc,
        })
    return in_maps


def run(in_maps, trace=False):
    nc = _get_nc()
    res = bass_utils.run_bass_kernel_spmd(nc, in_maps,
                                          core_ids=list(range(NCORES)),
                                          trace=trace)
    outp = np.concatenate([res.results[g]["out"] for g in range(NCORES)],
                          axis=0).astype(np.float32)
    return outp, res


def kernel(x, samples, W1, b1, W2, b2, Wp, bp):
    in_maps = make_in_maps(x, samples, W1, b1, W2, b2, Wp, bp)
    outp, _ = run(in_maps, trace=False)
    return outp


# revision 34
# speedup vs baseline: 1.1312x; 1.1312x over previous
"""AnchorDML Trainium2 kernel: 8-core SPMD, data-parallel over x rows with
sharded anchor encoding + AllGather of encoded anchors.

Problem (hardcoded):
    N, M, D, C = 8192, 4096, 512, 100
    xe = mish(mish(x @ W1 + b1) @ W2 + b2)          [N, D]
    se = mish(mish(samples @ W1 + b1) @ W2 + b2)    [M, D]
    dist = sqrt(max(|xe|^2 + |se|^2 - 2 xe@se.T, 0))  [N, M]
    out = log_softmax(tanh(dist @ Wp + bp), axis=1)   [N, C]

Sharding: core g handles x rows [1024g, 1024(g+1)) and encodes anchors
[512g, 512(g+1)); encoded (scaled) anchors + |se|^2 are AllGathered.

Design (trace-driven, vs the 213us baseline):
 - fp8(e4m3) distance GEMM in DoubleRow perf mode: seA and xe are stored
   as fp8 [128, 4k, cols]; each d2 tile is 2 matmuls contracting 256.
   |xe|^2 / |se|^2 are computed FROM the fp8-rounded values so d2 is
   exactly the squared distance of the fp8-perturbed points.
 - one AllGather moves [512, 258]-bf16 per rank: cols 0:256 hold the fp8
   seA pairs, cols 256:258 hold anchor r's fp32 |se|^2/... packed per
   row, so a single collective carries both and every gathered-side
   access pattern stays <= 3 dims.
 - a tiny warmup AllGather triggers ~10us in: it resolves the CC entry
   barrier and wakes the ncfw firmware, which measured 2.6x faster
   transfer + less start delay on the real collective.
 - the anchor->trigger chain is kept minimal: eTs/W1 loads head two DMA
   queues, later loads (W2, eTx, Wp) are semaphore-gated so they don't
   steal HBM bandwidth, sqse runs on ACT's Square (co-resident with
   Tanh: no table load), and dep helpers keep all anchor work ahead of
   x-side work on every engine.
 - x-side layer 2 is split into 512-row halves so the first half's xe
   and |xe|^2 are ready when the collective lands.
 - the main loop is rc-major (each half sweeps all 32 anchor tiles,
   2 fp8 DoubleRow matmuls each); the bias add drains PSUM into SBUF on
   DVE so the ACT sqrt pass is decoupled from the PE stream; each
   half's perceptron PSUM + transposes overlap the other half's sweep.
"""
import numpy as np
import ml_dtypes
from concourse import bass, bacc, tile, mybir, bass_utils, masks

N, M, D, C = 8192, 4096, 512, 100
NCORES = 8
RPC = N // NCORES      # 1024 x-rows per core
MPC = M // NCORES      # 512 anchors encoded per core
KD = D // 128          # 4 contraction chunks of 128
NMT = M // 128         # 32 anchor tiles in the distance matmul
NRC = RPC // 512       # 2 row-chunks of 512
TPG = MPC // 128       # 4 anchor tiles per gathered rank
AGW = MPC // 2         # ag row width in bf16 units (fp8 pairs)
AGR = D + 4            # 512 fp8 anchor rows + 4 rows of packed fp32 s2
LAG = 3                # zT matmul trails the d2 tiles by LAG tiles

F32 = mybir.dt.float32
F32R = mybir.dt.float32r
BF16 = mybir.dt.bfloat16
FP8 = mybir.dt.float8e4
AF = mybir.ActivationFunctionType
ALU = mybir.AluOpType
DR = mybir.MatmulPerfMode.DoubleRow


def _patched_tables(arch):
    """Subset the ACT table sets (keeping dict order — act_func_set_id is
    positional) so Exp/Ln resolve only to natural_log_exp_and_others and
    Tanh only to exp_and_others. The default first-match choice alternates
    exp_and_others <-> natural_log on every exp/ln pair, paying a 1.3us
    table load each time."""
    from concourse.hw_specs import get_activation_tables as orig
    out = {}
    for name, s in orig(arch).items():
        s = set(s)
        if name != "natural_log_exp_and_others":
            s.discard(AF.Exp)
            s.discard(AF.Ln)
            # keep Copy co-resident with Exp/Ln so the startup const
            # copies don't force an extra table load before the encoder
            if name != "sqrt_and_others":
                s.discard(AF.Copy)
        if name != "exp_and_others":
            s.discard(AF.Tanh)
            s.discard(AF.Square)
        out[name] = s
    return out


def build_kernel():
    bacc.get_activation_tables = _patched_tables
    nc = bacc.Bacc("TRN2", target_bir_lowering=False, debug=False,
                   num_devices=NCORES)

    eT = nc.dram_tensor("eT", [D, MPC + RPC], BF16, kind="ExternalInput")
    W1 = nc.dram_tensor("W1", [D, D], BF16, kind="ExternalInput")
    W2 = nc.dram_tensor("W2", [D, D], BF16, kind="ExternalInput")
    b1 = nc.dram_tensor("b1", [D, 1], F32, kind="ExternalInput")
    b2 = nc.dram_tensor("b2", [D, 1], F32, kind="ExternalInput")
    Wp = nc.dram_tensor("Wp", [M, C], F32, kind="ExternalInput")
    bp = nc.dram_tensor("bp", [1, C], F32, kind="ExternalInput")
    out = nc.dram_tensor("out", [RPC, C], F32, kind="ExternalOutput")

    with tile.TileContext(nc) as tc:
        _body(tc, eT, W1, W2, b1, b2, Wp, bp, out)

    nc.compile()
    return nc


def _body(tc, eT, W1, W2, b1, b2, Wp, bp, out):
    nc = tc.nc
    with (
        tc.tile_pool(name="const", bufs=1) as const,
        tc.tile_pool(name="wpool", bufs=1) as wpool,
        tc.tile_pool(name="spool", bufs=1) as spool,
        tc.tile_pool(name="xpool", bufs=1) as xpool,
        tc.tile_pool(name="gpool", bufs=1) as gpool,
        tc.tile_pool(name="mpool", bufs=2) as mpool,
        tc.tile_pool(name="dpool", bufs=1) as dpool,
        tc.tile_pool(name="zpool", bufs=2) as zpool,
        tc.tile_pool(name="ps", bufs=1, space="PSUM") as ps,
        tc.tile_pool(name="psz", bufs=1, space="PSUM") as psz,
        tc.tile_pool(name="dram", bufs=1, space="DRAM") as dram,
    ):
        # ---- warmup collective staging heads the pool queue (128B, in
        # flight by ~10us even under input-load traffic) ----
        warm_sb = const.tile([1, 64], BF16)
        nc.gpsimd.memset(warm_sb[:], 1.0)
        warm_in = dram.tile([1, 64], BF16)
        warm_out = dram.tile([NCORES, 64], BF16, addr_space="Shared")
        nc.gpsimd.dma_start(warm_in[:], warm_sb[:])
        nc.gpsimd.collective_compute(
            "AllGather", ALU.bypass,
            replica_groups=[list(range(NCORES))],
            ins=[warm_in.opt()], outs=[warm_out.opt()])

        # ---- input loads: the two tensors gating the first anchor
        # matmul (eTs, W1) head the sync and pool queues so their ~1MB
        # gets the full HBM bandwidth; W2/eTx/Wp issue is gated (helpers
        # added below) until the anchor encode is underway. ----
        eTs_sb = spool.tile([128, KD, MPC], BF16)
        for h in range(2):
            nc.sync.dma_start(
                eTs_sb[:, 2 * h:2 * h + 2, :],
                eT[256 * h:256 * (h + 1), :MPC].rearrange(
                    "(k p) m -> p k m", p=128))
        W1_sb = wpool.tile([128, KD, D], BF16)
        for h in range(2):
            nc.gpsimd.dma_start(
                W1_sb[:, 2 * h:2 * h + 2, :],
                W1[256 * h:256 * (h + 1), :].rearrange(
                    "(k p) d -> p k d", p=128))
        b1c_sb = wpool.tile([128, KD], F32)
        nc.gpsimd.dma_start(b1c_sb[:],
                            b1[:].rearrange("(k p) o -> p (k o)", p=128))
        b2c_sb = wpool.tile([128, KD], F32)
        nc.gpsimd.dma_start(b2c_sb[:],
                            b2[:].rearrange("(k p) o -> p (k o)", p=128))

        W2_sb = wpool.tile([128, KD, D], BF16)
        w2_dma = nc.sync.dma_start(W2_sb[:, :, :],
                                   W2[:].rearrange("(k p) d -> p k d", p=128))
        eTx_sb = xpool.tile([128, KD, RPC], BF16)
        etx_dmas = []
        for rc in range(NRC):
            etx_dmas.append(nc.sync.dma_start(
                eTx_sb[:, :, 512 * rc:512 * (rc + 1)],
                eT[:, MPC + 512 * rc:MPC + 512 * (rc + 1)].rearrange(
                    "(k p) m -> p k m", p=128)))

        # ---- constants (pool queue, before the gated Wp load) ----
        ident = const.tile([C, C], F32)
        masks.make_identity(nc, ident[:])
        ones_col = const.tile([128, 1], BF16)    # lhsT for row-sum matmuls
        nc.gpsimd.memset(ones_col[:], 1.0)
        onesr_f32 = const.tile([1, 512], F32)
        nc.gpsimd.memset(onesr_f32[:], 1.0)
        ones512 = const.tile([1, 512], F32R)     # rhs/lhsT for rank-1 terms
        nc.scalar.activation(ones512[:], onesr_f32[:], AF.Copy)
        bp_sb = wpool.tile([1, C], F32R)
        nc.gpsimd.dma_start(bp_sb[:], bp[:].bitcast(F32R))

        Wp_sb = wpool.tile([128, NMT, C], F32R)
        wp_dma = nc.gpsimd.dma_start(
            Wp_sb[:, :, :],
            Wp[:].bitcast(F32R).rearrange("(t p) c -> p t c", p=128))

        W1_ks = [W1_sb[:, k, :] for k in range(KD)]
        W2_ks = [W2_sb[:, k, :] for k in range(KD)]
        eTs_ks = [eTs_sb[:, k, :] for k in range(KD)]
        eTx_ks = [eTx_sb[:, k, :] for k in range(KD)]

        def enc_phase(dst, dst_off, Wks, bcol, src_ks, src_off, width,
                      vscale=None):
            """dst[:, :, dst_off:dst_off+width] = mish(src.T @ W + b) in
            dst's dtype (bf16 hidden / fp8 final). v is staged (with bias)
            so psum recycles fast; sp=ln(1+e^v) lands in a bf16 temp; tanh
            + the v*t multiply are batched over the whole phase."""
            nw = width // 512
            first_exp, first_vadd, first_mm = [], [], []
            vstage = mpool.tile([128, KD, width], BF16, tag="vstage")
            tstage = mpool.tile([128, KD, width], BF16, tag="tstage")
            for w in range(nw):
                ssl = slice(src_off + 512 * w, src_off + 512 * (w + 1))
                for f in range(KD):
                    vps = ps.tile([128, 512], F32, tag="mm", bufs=4)
                    for k in range(KD):
                        mm_i = nc.tensor.matmul(vps[:],
                                                Wks[k][:, 128 * f:128 * (f + 1)],
                                                src_ks[k][:, ssl],
                                                start=(k == 0),
                                                stop=(k == KD - 1))
                        if not first_mm:
                            first_mm.append(mm_i)
                    u = mpool.tile([128, 512], BF16, tag="mu", bufs=4)
                    e_i = nc.scalar.activation(u[:], vps[:], AF.Exp,
                                               bias=bcol[:, f:f + 1])
                    if not first_exp:
                        first_exp.append(e_i)
                    if vscale is None:
                        v_i = nc.vector.tensor_scalar_add(
                            vstage[:, f, 512 * w:512 * (w + 1)], vps[:],
                            bcol[:, f:f + 1])
                    else:
                        v_i = nc.vector.tensor_scalar(
                            vstage[:, f, 512 * w:512 * (w + 1)], vps[:],
                            bcol[:, f:f + 1], vscale,
                            op0=ALU.add, op1=ALU.mult)
                    if not first_vadd:
                        first_vadd.append(v_i)
                    nc.scalar.activation(tstage[:, f, 512 * w:512 * (w + 1)],
                                         u[:], AF.Ln, bias=1.0)
            dsl = slice(dst_off, dst_off + width)
            tanh_i = nc.scalar.activation(tstage[:, :, :width],
                                          tstage[:, :, :width], AF.Tanh)
            mult_i = nc.vector.tensor_tensor(dst[:, :, dsl],
                                             vstage[:, :, :width],
                                             tstage[:, :, :width], op=ALU.mult)
            return {"first_exp": first_exp[0], "first_vadd": first_vadd[0],
                    "first_mm": first_mm[0], "tanh": tanh_i, "mult": mult_i}

        # ---- anchor columns first, both layers, so the AllGather can be
        # issued as early as possible; the -2 distance scaling is folded
        # into the layer-2 mish multiply so it emits seA = -2*se (exact
        # in fp8: power-of-2 scale) ----
        h_se = spool.tile([128, KD, MPC], BF16)
        hse_ks = [h_se[:, k, :] for k in range(KD)]
        se1_h = enc_phase(h_se, 0, W1_ks, b1c_sb, eTs_ks, 0, MPC)
        seA_sb = spool.tile([128, KD, MPC], FP8)
        se2_h = enc_phase(seA_sb, 0, W2_ks, b2c_sb, hse_ks, 0, MPC,
                          vscale=-2.0)
        # gate the bandwidth-hungry loads behind the first anchor matmul
        tile.add_dep_helper(w2_dma.ins, se1_h["first_mm"].ins, sync=True,
                            reason="W2 issue after anchor L1 starts")
        for e in etx_dmas:
            tile.add_dep_helper(e.ins, se1_h["first_mm"].ins, sync=True,
                                reason="eTx issue after anchor L1 starts")
        tile.add_dep_helper(wp_dma.ins, se2_h["first_mm"].ins, sync=True,
                            reason="Wp issue after anchor L2 starts")

        # s2 row = sum_d seA^2 / 4 (from the fp8-rounded values, so d2 is
        # exactly the squared distance of the perturbed points). Square
        # runs on ACT (co-resident with Tanh: no table load).
        sqse_sb = spool.tile([128, KD, MPC], BF16, tag="h_se")
        sq_se_i = nc.scalar.activation(sqse_sb[:, :, :], seA_sb[:, :, :],
                                       AF.Square)
        s2ps = ps.tile([1, 512], F32, tag="tr", bufs=2)
        for k in range(KD):
            s2_mm_i = nc.tensor.matmul(s2ps[:], ones_col[:], sqse_sb[:, k, :],
                                       start=(k == 0), stop=(k == KD - 1))
        s2row_sb = spool.tile([1, MPC], F32)
        s2row_i = nc.vector.tensor_scalar_mul(s2row_sb[:], s2ps[:], 0.25)

        # one collective: [seA (fp8 as bf16 pairs, 512 rows) ; s2 (fp32
        # packed as 4 rows)] — row-appended so the gathered per-rank seA
        # reads stay fully contiguous (a strided row read costs ~6us of
        # descriptor-issue time)
        ag_in = dram.tile([AGR, AGW], BF16)
        ag_out = dram.tile([NCORES * AGR, AGW], BF16, addr_space="Shared")
        nc.sync.dma_start(
            ag_in[:D, :].rearrange("(k p) m -> p k m", p=128),
            seA_sb[:, :, :].bitcast(BF16))
        nc.sync.dma_start(
            ag_in[D:AGR, :].rearrange("(o a) m -> o (a m)", o=1),
            s2row_sb[:].bitcast(BF16))
        nc.gpsimd.collective_compute(
            "AllGather", ALU.bypass,
            replica_groups=[list(range(NCORES))],
            ins=[ag_in.opt()], outs=[ag_out.opt()])

        # ---- x columns overlap the AllGather, both layers split into
        # 512-row halves so the first half's xe and |xe|^2 are ready the
        # moment the collective lands. Ordering hints keep the
        # anchor->AllGather chain ahead of x-side work on every engine. ----
        h_xe = xpool.tile([128, KD, RPC], BF16)
        hxe_ks = [h_xe[:, k, :] for k in range(KD)]
        xe_sb = xpool.tile([128, KD, RPC], FP8)
        x2row_sb = xpool.tile([1, RPC], F32R)
        x2b_sb = xpool.tile([128, RPC], F32)
        def x2_phase(rc):
            """|xe|^2 row + broadcast tile for one 512-row half."""
            rsl = slice(512 * rc, 512 * (rc + 1))
            sqxe = xpool.tile([128, KD, 512], BF16, tag=f"sqxe{rc}")
            nc.vector.tensor_tensor(sqxe[:, :, :], xe_sb[:, :, rsl],
                                    xe_sb[:, :, rsl], op=ALU.mult)
            xps = ps.tile([1, 512], F32, tag="tr", bufs=2)
            for k in range(KD):
                nc.tensor.matmul(xps[:], ones_col[:], sqxe[:, k, :],
                                 start=(k == 0), stop=(k == KD - 1))
            nc.vector.tensor_copy(x2row_sb[:, rsl], xps[:])
            xbs = ps.tile([128, 512], F32, tag="mm", bufs=4)
            nc.tensor.matmul(xbs[:], ones512[:, :128], x2row_sb[:, rsl],
                             start=True, stop=True)
            nc.vector.tensor_copy(x2b_sb[:, rsl], xbs[:])

        xe1_h = enc_phase(h_xe, 0, W1_ks, b1c_sb, eTx_ks, 0, RPC)
        xe2_h = enc_phase(xe_sb, 0, W2_ks, b2c_sb, hxe_ks, 0, RPC)
        x2_phase(0)
        x2_phase(1)
        tile.add_dep_helper(xe1_h["first_exp"].ins, sq_se_i.ins, sync=False,
                            reason="anchor ACT chain before x-side ACT")
        tile.add_dep_helper(xe1_h["first_vadd"].ins, s2row_i.ins, sync=False,
                            reason="anchor DVE chain before x-side DVE")
        tile.add_dep_helper(xe1_h["first_mm"].ins, s2_mm_i.ins, sync=False,
                            reason="anchor PE chain + s2 sums before x-side PE")

        # ---- load gathered anchors: per-rank seA descriptors alternate
        # between the sync and pool queues (multiple descriptors spread
        # across DMA engines; a single big one serializes on one engine).
        # The tiny s2 descriptor and rank 0 head the queues: they gate
        # the first distance tile. ----
        s2c_sb = gpool.tile([128, NCORES, TPG], F32)
        seAg_sb = gpool.tile([128, NCORES, KD, MPC], FP8)
        for g in range(NCORES):
            q = nc.sync if g % 2 == 0 else nc.gpsimd
            descs = [
                (s2c_sb[:, g, :],
                 ag_out[AGR * g + D:AGR * (g + 1), :].bitcast(F32)
                 .rearrange("a p -> p a")),
                (seAg_sb[:, g, :, :].bitcast(BF16),
                 ag_out[AGR * g:AGR * g + D, :].rearrange(
                     "(k p) m -> p k m", p=128)),
            ]
            # rank 0 gates the first tile: its s2 first; later ranks'
            # (bigger) seA payloads head their queue slots instead
            for dst, src in (descs if g < 2 else descs[::-1]):
                q.dma_start(dst, src)

        # ---- main fused loop over anchor tiles; both row-chunks share
        # each tile's DoubleRow weights. The zT matmul for tile t is
        # emitted after the d2 group of tile t+LAG so the in-order PE
        # stream never waits on the sqrt pass. ----
        zt_ps = [psz.tile([C, 512], F32, name=f"ztps{rc}") for rc in range(NRC)]
        for rc in range(NRC):
            nc.tensor.matmul(zt_ps[rc][:], bp_sb[:], ones512[:],
                             start=True, stop=False, skip_group_check=True)
        dist_tiles = {}
        first_sqrt = []
        for t in range(NMT):
            g, tl = divmod(t, TPG)
            for rc in range(NRC):
                rsl = slice(512 * rc, 512 * (rc + 1))
                d2ps = ps.tile([128, 512], F32, tag="mm", bufs=4)
                for q in range(2):
                    nc.tensor.matmul(
                        d2ps[:],
                        seAg_sb[:, g, 2 * q:2 * q + 2, 128 * tl:128 * (tl + 1)],
                        xe_sb[:, 2 * q:2 * q + 2, rsl],
                        start=(q == 0), stop=(q == 1), perf_mode=DR)
                # d2 += s2[m] (per-partition) + x2[r] (broadcast row), fp32
                nc.vector.scalar_tensor_tensor(
                    d2ps[:], d2ps[:], s2c_sb[:, g, tl:tl + 1],
                    x2b_sb[:, rsl], op0=ALU.add, op1=ALU.add)
                distT = dpool.tile([128, 512], F32R, tag="dist", bufs=6)
                sq_i = nc.scalar.activation(distT[:], d2ps[:], AF.Sqrt)
                if not first_sqrt:
                    first_sqrt.append(sq_i)
                    tile.add_dep_helper(
                        sq_i.ins, xe2_h["tanh"].ins, sync=False,
                        reason="sqrt table load after the last encoder ACT")
                dist_tiles[(t, rc)] = distT
            if t >= LAG:
                for rc in range(NRC):
                    nc.tensor.matmul(zt_ps[rc][:], Wp_sb[:, t - LAG, :],
                                     dist_tiles.pop((t - LAG, rc))[:],
                                     start=False, stop=False,
                                     skip_group_check=True)
        for t in range(NMT - LAG, NMT):
            for rc in range(NRC):
                nc.tensor.matmul(zt_ps[rc][:], Wp_sb[:, t, :],
                                 dist_tiles.pop((t, rc))[:],
                                 start=False, stop=(t == NMT - 1),
                                 skip_group_check=True)

        # ---- epilogue: bias'd zT -> sbuf, transpose, then one batched
        # tanh + log-softmax pass (tanh output is in [-1,1] so no
        # max-subtraction is needed) ----
        zpre_sb = zpool.tile([128, 2 * NRC * 2, C], BF16, bufs=1)
        for rc in range(NRC):
            zt_sb = zpool.tile([C, 512], F32, bufs=2, tag="ztsb")
            nc.vector.tensor_copy(zt_sb[:], zt_ps[rc][:])
            for j in range(4):
                ztr = ps.tile([128, C], F32, tag="tr", bufs=2)
                nc.tensor.matmul(ztr[:], zt_sb[:, 128 * j:128 * (j + 1)],
                                 ident[:], is_transpose=True)
                nc.vector.tensor_copy(zpre_sb[:, 4 * rc + j, :], ztr[:])
        NT = 2 * NRC * 2  # 8 tiles of 128 rows
        zth_sb = zpool.tile([128, NT, C], BF16, bufs=1)
        nc.scalar.activation(zth_sb[:, :, :], zpre_sb[:, :, :], AF.Tanh)
        e_sb = zpool.tile([128, NT, C], BF16, bufs=1, tag="zpre_sb")
        nc.scalar.activation(e_sb[:, :, :], zth_sb[:, :, :], AF.Exp)
        ssum = zpool.tile([128, NT], F32, bufs=1)
        nc.vector.tensor_reduce(ssum[:], e_sb[:, :, :],
                                axis=mybir.AxisListType.X, op=ALU.add)
        lns = zpool.tile([128, NT], F32, bufs=1)
        nc.scalar.activation(lns[:], ssum[:], AF.Ln)
        for rc in range(NRC):
            o_sb = zpool.tile([128, 4, C], F32, bufs=2, tag="osb")
            for j in range(4):
                jj = 4 * rc + j
                nc.vector.tensor_scalar(o_sb[:, j, :], zth_sb[:, jj, :],
                                        lns[:, jj:jj + 1], None,
                                        op0=ALU.subtract)
            nc.sync.dma_start(
                out[512 * rc:512 * (rc + 1), :].rearrange(
                    "(j p) c -> p j c", p=128),
                o_sb[:, :, :])


_NC_CACHE = None


def _get_nc():
    global _NC_CACHE
    if _NC_CACHE is None:
        _NC_CACHE = build_kernel()
    return _NC_CACHE


def make_in_maps(x, samples, W1, b1, W2, b2, Wp, bp):
    bf = ml_dtypes.bfloat16
    x = np.asarray(x, dtype=np.float32)
    samples = np.asarray(samples, dtype=np.float32)
    W1b = np.ascontiguousarray(np.asarray(W1, dtype=np.float32).astype(bf))
    W2b = np.ascontiguousarray(np.asarray(W2, dtype=np.float32).astype(bf))
    Wpc = np.ascontiguousarray(np.asarray(Wp, dtype=np.float32))
    b1c = np.ascontiguousarray(np.asarray(b1, dtype=np.float32).reshape(D, 1))
    b2c = np.ascontiguousarray(np.asarray(b2, dtype=np.float32).reshape(D, 1))
    bpc = np.ascontiguousarray(np.asarray(bp, dtype=np.float32).reshape(1, C))
    in_maps = []
    for g in range(NCORES):
        sT_g = samples[MPC * g:MPC * (g + 1), :].T
        xT_g = x[RPC * g:RPC * (g + 1), :].T
        eT_g = np.concatenate([sT_g, xT_g], axis=1).astype(bf)
        in_maps.append({
            "eT": np.ascontiguousarray(eT_g),
            "W1": W1b, "W2": W2b, "b1": b1c, "b2": b2c,
            "Wp": Wpc, "bp": bpc,
        })
    return in_maps


def run(in_maps, trace=False):
    nc = _get_nc()
    res = bass_utils.run_bass_kernel_spmd(nc, in_maps,
                                          core_ids=list(range(NCORES)),
                                          trace=trace)
    outp = np.concatenate([res.results[g]["out"] for g in range(NCORES)],
                          axis=0).astype(np.float32)
    return outp, res


def kernel(x, samples, W1, b1, W2, b2, Wp, bp):
    in_maps = make_in_maps(x, samples, W1, b1, W2, b2, Wp, bp)
    outp, _ = run(in_maps, trace=False)
    return outp
